# revision 42
# baseline (speedup 1.0000x reference)
"""Trainium2 Bass kernel for nn_ChunkedAttention (B=4, T=4096, D=1024, H=16, dh=64).

Sharding: 8 cores = 4 batches x 2 head-groups (8 heads each). Host sums the
two partial out-projections per batch in fp32.

All inputs bf16, pre-shuffled host-side to [partition, chunk, free] so every
DMA is 128 contiguous 8KB descriptors. The attention inner loop runs in
2-kt blocks: both k-tiles' row-tiled (banded K=64) score pairs issue
back-to-back — banded pairs overlap on the PE and the ~100ns tile-mode
entry cost is paid once per block — then the previous block's four
full-array AV matmuls (exit cost paid once). Projection/out-projection work
is queued as 2-matmul fill pieces with deadlines (proj(t) before
attention(t), outproj(q) before attention(q+2)) and drained into the PE's
spare cycles between a block's scores and AVs, so the softmax (Act) engine
is never starved by a bulk projection phase and leftovers roll into later
(larger) groups' slack. The softmax denominator rides the AV matmuls as a
65th ones-column; reciprocals use the fast approximate DVE op; the last
group normalizes per head pair immediately so the final out-projection
isn't serialized behind the whole group.
"""

import os
import sys

import numpy as np

for _p in ("/opt/trn_rl_repo",):
    if _p not in sys.path and os.path.isdir(_p):
        sys.path.insert(0, _p)

import ml_dtypes

import concourse.bass as bass
import concourse.mybir as mybir
import concourse.tile as tile
from concourse.bacc import Bacc
from concourse.bass_utils import run_bass_kernel_spmd

F32 = mybir.dt.float32
BF16 = mybir.dt.bfloat16
EXP = mybir.ActivationFunctionType.Exp
MULT = mybir.AluOpType.mult

B, T, D = 4, 4096, 1024
HG = 512          # head-group width per core (8 heads x 64)
NH, DH = 8, 64    # heads per core, head dim
NPAIR = 4         # head pairs per core
QG = 512          # query-group width
NQG = T // QG     # 8
NKT = T // 128    # 32 k-tiles
NDC = D // 128    # 8 d_model chunks
SCALE = 1.0 / np.sqrt(DH)  # 0.125

BF = ml_dtypes.bfloat16


def build_nc():
    nc = Bacc()
    # x^T pre-shuffled host-side to [tcn, p, dc, u] so each per-chunk DMA is
    # 128 descriptors of 8KB (contiguous per partition) instead of 1024x1KB
    xT_d = nc.dram_tensor("xT", [NQG, 128, NDC, QG], BF16, kind="ExternalInput")
    # weights pre-shuffled host-side to [p, dc, h] so each DMA is 128
    # descriptors of 8KB (contiguous per partition) instead of 1024x1KB
    wq_d = nc.dram_tensor("wq", [128, NDC, HG], BF16, kind="ExternalInput")
    wk_d = nc.dram_tensor("wk", [128, NDC, HG], BF16, kind="ExternalInput")
    wv_d = nc.dram_tensor("wv", [128, NDC, HG], BF16, kind="ExternalInput")
    wo_d = nc.dram_tensor("wo", [128, NPAIR, D], BF16, kind="ExternalInput")
    tri_d = nc.dram_tensor("tri", [128, 128], BF16, kind="ExternalInput")
    y_d = nc.dram_tensor("y", [T, D], BF16, kind="ExternalOutput")

    with tile.TileContext(nc) as tc_:
        with (
            tc_.tile_pool(name="const", bufs=1) as pconst,
            tc_.tile_pool(name="pxt", bufs=2) as pxt,
            tc_.tile_pool(name="pq", bufs=3) as pq,
            tc_.tile_pool(name="pet", bufs=4) as pet,
            tc_.tile_pool(name="pnrm", bufs=2) as pnrm,
            tc_.tile_pool(name="prb", bufs=3) as prb,
            tc_.tile_pool(name="psS", bufs=2, space="PSUM") as psS,
            tc_.tile_pool(name="psAV", bufs=2, space="PSUM") as psAV,
            tc_.tile_pool(name="psM", bufs=2, space="PSUM") as psM,
        ):
            kt_sb = pconst.tile([128, NPAIR, T], BF16, tag="kt")
            v_sb = pconst.tile([128, NKT, NH, DH + 1], BF16, tag="v")
            tri_sb = pconst.tile([128, 128], BF16, tag="tri")
            wq_sb = pconst.tile([128, NDC, HG], BF16, tag="wq")
            wk_sb = pconst.tile([128, NDC, HG], BF16, tag="wk")
            wv_sb = pconst.tile([128, NDC, HG], BF16, tag="wv")
            wo_sb = pconst.tile([128, NPAIR, D], BF16, tag="wo")

            def load_xt_first(tcn):
                # split into 2-dc parts so the first proj pieces (which read
                # 2 dc each) start after ~0.25MB lands, not the full 1MB
                xt = pxt.tile([128, NDC, QG], BF16, tag="xt", name="xt")
                for d0 in range(0, NDC, 2):
                    nc.sync.dma_start(
                        xt[:, d0 : d0 + 2, :], xT_d[tcn, :, d0 : d0 + 2, :]
                    )
                return xt

            # startup DMA priority: the 5MB of inputs are HBM-bandwidth
            # bound (~12us), so emit only what the first q/k chains need
            # first; wv/tri/wo descriptors queue behind them
            xt0 = load_xt_first(0)
            for d0 in range(0, NDC, 2):
                nc.sync.dma_start(
                    wq_sb[:, d0 : d0 + 2, :], wq_d[:, d0 : d0 + 2, :]
                )
            nc.sync.dma_start(wk_sb[:], wk_d[:])
            nc.gpsimd.memset(v_sb[:, :, :, DH : DH + 1], 1.0)

            def load_late_consts():
                nc.sync.dma_start(wv_sb[:], wv_d[:])
                nc.sync.dma_start(tri_sb[:], tri_d[:])
                nc.sync.dma_start(wo_sb[:], wo_d[:])

            def load_xt(tcn):
                xt = pxt.tile([128, NDC, QG], BF16, tag="xt", name="xt")
                nc.sync.dma_start(xt[:], xT_d[tcn])
                return xt

            # chains are emitted as 2-matmul pieces (~430ns of PE each) so
            # the fill queue never blocks the PE long enough to starve the
            # Act engine between score matmuls
            def q_chain_pieces(tcn, xt, qst, hp):
                st = {}

                def piece(d0):
                    if d0 == 0:
                        st["ps"] = psM.tile([128, QG], F32, tag="mm", name="pq_ps")
                    for dc in range(d0, d0 + 2):
                        nc.tensor.matmul(
                            st["ps"][:],
                            wq_sb[:, dc, hp * 128 : (hp + 1) * 128],
                            xt[:, dc, :],
                            start=(dc == 0),
                            stop=(dc == NDC - 1),
                        )
                    if d0 == NDC - 2:
                        nc.vector.tensor_copy(qst[:, hp, :], st["ps"][:])

                return [lambda d0=d0: piece(d0) for d0 in range(0, NDC, 2)]

            def k_chain_pieces(tcn, xt, hp):
                st = {}

                def piece(d0):
                    if d0 == 0:
                        st["ps"] = psM.tile([128, QG], F32, tag="mm", name="pk_ps")
                    for dc in range(d0, d0 + 2):
                        nc.tensor.matmul(
                            st["ps"][:],
                            wk_sb[:, dc, hp * 128 : (hp + 1) * 128],
                            xt[:, dc, :],
                            start=(dc == 0),
                            stop=(dc == NDC - 1),
                        )
                    if d0 == NDC - 2:
                        nc.vector.tensor_copy(
                            kt_sb[:, hp, tcn * QG : (tcn + 1) * QG], st["ps"][:]
                        )

                return [lambda d0=d0: piece(d0) for d0 in range(0, NDC, 2)]

            def v_chain_pieces(tcn, xt, ts):
                st = {}

                def piece(d0):
                    if d0 == 0:
                        st["ps"] = psM.tile([128, QG], F32, tag="mm", name="pv_ps")
                    for dc in range(d0, d0 + 2):
                        nc.tensor.matmul(
                            st["ps"][:],
                            xt[:, dc, ts * 128 : (ts + 1) * 128],
                            wv_sb[:, dc, :],
                            start=(dc == 0),
                            stop=(dc == NDC - 1),
                        )
                    if d0 == NDC - 2:
                        ktg = tcn * 4 + ts
                        nc.vector.tensor_copy(
                            v_sb[:, ktg, :, 0:DH],
                            st["ps"].rearrange("p (h d) -> p h d", h=NH),
                        )

                return [lambda d0=d0: piece(d0) for d0 in range(0, NDC, 2)]

            def proj_pieces(tcn, xt, qst):
                items = []
                for hp in range(NPAIR):
                    items.extend(q_chain_pieces(tcn, xt, qst, hp))
                for hp in range(NPAIR):
                    items.extend(k_chain_pieces(tcn, xt, hp))
                for ts in range(4):
                    items.extend(v_chain_pieces(tcn, xt, ts))
                return items

            def op_pieces(qg, mrg, qc, half, odts=None):
                st = {}

                def finish(ps):
                    yt = prb.tile([128, 512], BF16, tag="yt")
                    nc.vector.tensor_copy(yt[:], ps[:])
                    nc.sync.dma_start(
                        y_d[
                            qg * QG + qc * 128 : qg * QG + (qc + 1) * 128,
                            half * 512 : (half + 1) * 512,
                        ],
                        yt[:],
                    )

                def piece(h0):
                    if h0 == 0:
                        st["ps"] = psM.tile([128, 512], F32, tag="mm", name="op")
                    for hp in range(h0, h0 + 2):
                        nc.tensor.matmul(
                            st["ps"][:],
                            mrg[:, hp, qc * 128 : (qc + 1) * 128],
                            wo_sb[:, hp, half * 512 : (half + 1) * 512],
                            start=(hp == 0),
                            stop=(hp == NPAIR - 1),
                        )
                    if h0 == NPAIR - 2:
                        finish(st["ps"])

                def piece_banded(h0):
                    # last group: the odd head's normalized output never got
                    # the partition-shift DMA into mrg rows 64-127; contract
                    # it from partitions 0-63 as a K=64 row tile at array
                    # rows 64-127 instead (keeps the final out-projection
                    # off the DMA latency path)
                    if h0 == 0:
                        st["ps"] = psM.tile([128, 512], F32, tag="mm", name="op")
                    for hp in range(h0, h0 + 2):
                        nc.tensor.matmul(
                            st["ps"][:],
                            mrg[0:64, hp, qc * 128 : (qc + 1) * 128],
                            wo_sb[0:64, hp, half * 512 : (half + 1) * 512],
                            start=(hp == 0),
                            stop=False,
                            tile_position=(0, 0),
                        )
                        nc.tensor.matmul(
                            st["ps"][:],
                            odts[hp][:, qc * 128 : (qc + 1) * 128],
                            wo_sb[64:128, hp, half * 512 : (half + 1) * 512],
                            start=False,
                            stop=(hp == NPAIR - 1),
                            tile_position=(64, 0),
                        )
                    if h0 == NPAIR - 2:
                        finish(st["ps"])

                pc = piece_banded if odts is not None else piece
                return [lambda h0=h0: pc(h0) for h0 in range(0, NPAIR, 2)]

            def outproj_pieces(qg, mrg, odts=None):
                odts = odts or None  # only the last group populates it
                items = []
                for qc in range(4):
                    for half in range(2):
                        items.extend(op_pieces(qg, mrg, qc, half, odts))
                return items

            # deferred PE work as (deadline, fn): fn must run before
            # attention(deadline) starts (proj(t) feeds attention(t);
            # outproj(q) must finish before mrg(q)'s pool slot recycles at
            # attention(q+2)). Leftovers roll forward into later groups'
            # PE slack instead of stalling the Act engine in a bulk drain.
            fill = []

            def normalize_hp(qg, hp, av, av_all, mrg, odts):
                """Normalize one head pair with a fast approximate
                reciprocal (saves ~3.4us of iterative DVE divide per head
                pair on the final group's latency-critical tail)."""
                for j in (1, 0):  # j=1 first: its result rides a DMA, so
                    # start that leg of the chain as early as possible
                    idx = 2 * hp + j
                    rs0b = prb.tile([1, QG], BF16, tag="rs0b", bufs=2)
                    nc.sync.dma_start(rs0b[:], av_all[DH : DH + 1, idx, :])
                    rs0 = prb.tile([1, QG], F32, tag="rs0")
                    nc.vector.tensor_copy(rs0[:], rs0b[:])
                    nc.vector.reciprocal_approx_fast(rs0[:], rs0[:])
                    rb = prb.tile([DH, QG], F32, tag="rb", bufs=8)
                    nc.gpsimd.partition_broadcast(rb[:], rs0[:])
                    if j == 0:
                        nc.vector.tensor_tensor(
                            mrg[0:DH, hp, :], av_all[0:DH, idx, :], rb[:], MULT
                        )
                    else:
                        # (a banded K=64 out-projection from partitions 0-63
                        # at tile row 64 would skip this DMA, but walrus
                        # codegen rejects lhsT/tile_position partition
                        # mismatch — the partition shift must ride a DMA)
                        odt = prb.tile([DH, QG], BF16, tag="odt7", bufs=4)
                        nc.vector.tensor_tensor(
                            odt[:], av_all[0:DH, idx, :], rb[:], MULT
                        )
                        nc.sync.dma_start(mrg[DH:128, hp, :], odt[:])

            def drain_fill(deadline):
                while fill and fill[0][0] <= deadline:
                    fill.pop(0)[1]()

            def attention(qg, qst):
                """Causal attention + softmax normalize for query group qg.

                Drains the fill queue (projection pieces for future
                T-chunks, previous group's out-projection pieces) into the
                PE's spare per-kt cycles.
                """
                ktmax = 4 * (qg + 1)
                n_kt = NPAIR * ktmax
                odts = {}
                av_all = pnrm.tile([DH + 1, 2 * NPAIR, QG], BF16, tag="avsb")
                sum_sb = pnrm.tile([2 * NPAIR, QG], BF16, tag="sums")
                mrg = pnrm.tile([128, NPAIR, QG], BF16, tag="mrg")
                # pace fill pops over the group's kt iterations, starting a
                # couple of kts in (lets input DMAs land first); leftovers
                # roll to later (larger) groups
                kt_count = 0

                for hp in range(NPAIR):
                    av = [
                        psAV.tile([DH + 1, QG], F32, tag="av", name=f"av{j}")
                        for j in range(2)
                    ]

                    def emit_av(e_t, kt):
                        diag = kt - 4 * qg
                        dlt = 128 * diag if diag >= 0 else 0
                        for j in range(2):
                            nc.tensor.matmul(
                                av[j][:, dlt:],
                                v_sb[:, kt, 2 * hp + j, :],
                                e_t[:, j, dlt:],
                                start=(kt == 0),
                                stop=(kt == ktmax - 1),
                            )

                    # 2-kt blocks: both kts' banded score pairs issue
                    # back-to-back (bands overlap, and the ~100ns tile-mode
                    # entry cost is paid once per block, not per kt), then
                    # the previous block's 4 full-array AV matmuls (paying
                    # the banded->full exit cost once). exps of the current
                    # block run on Act while the previous block's AVs run
                    # on the PE.
                    pend = []
                    for ktb in range(0, ktmax, 2):
                        kts = (ktb, ktb + 1)
                        ets = []
                        for kt in kts:
                            diag = kt - 4 * qg
                            dlt = 128 * diag if diag >= 0 else 0
                            s_t = psS.tile([128, 2, QG], F32, tag="st")
                            for j in range(2):
                                nc.tensor.matmul(
                                    s_t[:, j, dlt:],
                                    kt_sb[
                                        64 * j : 64 * (j + 1),
                                        hp,
                                        kt * 128 : (kt + 1) * 128,
                                    ],
                                    qst[64 * j : 64 * (j + 1), hp, dlt:],
                                    start=True,
                                    stop=True,
                                    tile_position=(64 * j, 0),
                                )
                            ets.append((s_t, dlt, diag))
                        new_pend = []
                        for kt, (s_t, dlt, diag) in zip(kts, ets):
                            e_t = pet.tile([128, 2, QG], BF16, tag="exps")
                            nc.scalar.activation(
                                e_t[:, :, dlt:], s_t[:, :, dlt:], EXP,
                                scale=SCALE,
                            )
                            if diag >= 0:
                                for j in range(2):
                                    nc.vector.tensor_tensor(
                                        e_t[:, j, dlt : dlt + 128],
                                        e_t[:, j, dlt : dlt + 128],
                                        tri_sb[:],
                                        MULT,
                                    )
                            new_pend.append((e_t, kt))
                        kt_count += 2
                        if fill and kt_count >= (3 if qg >= 2 else 1):
                            # pop fill between the scores and the AVs: the
                            # banded->full transition cost is lower into a
                            # proj piece than into an AV, and the pend AVs
                            # have no consumer waiting on them
                            slots = n_kt - kt_count
                            npop = max(1, -(-(2 * len(fill)) // max(1, slots)))
                            for _ in range(min(npop, 4, len(fill))):
                                fill.pop(0)[1]()
                        for p in pend:
                            emit_av(*p)
                        pend = new_pend
                    # row end: the last exps have no following score work, so
                    # feed the PE fill pieces while the Act engine drains them
                    for _ in range(min(3, len(fill))):
                        fill.pop(0)[1]()
                    for p in pend:
                        emit_av(*p)
                    # drain PSUM to SBUF so the next head pair can accumulate
                    last = qg == NQG - 1
                    for j in range(2):
                        idx = 2 * hp + j
                        nc.vector.tensor_copy(av_all[:, idx, :], av[j][:])
                        if not last:
                            nc.sync.dma_start(
                                sum_sb[idx : idx + 1, :],
                                av_all[DH : DH + 1, idx, :],
                            )
                    if last:
                        # last group: normalize per head pair immediately so
                        # the final out-projection isn't serialized at the end
                        normalize_hp(qg, hp, av, av_all, mrg, odts)

                if qg < NQG - 1:
                    # batched reciprocal of all 8 softmax-sum rows, then
                    # broadcasts (GpSimd) overlapping multiplies (DVE)
                    rcp = pnrm.tile([2 * NPAIR, QG], F32, tag="rcp")
                    nc.vector.tensor_copy(rcp[:], sum_sb[:])
                    nc.vector.reciprocal_approx_fast(rcp[:], rcp[:])
                    rbs = []
                    for idx in range(2 * NPAIR):
                        rs0 = prb.tile([1, QG], F32, tag="rs0")
                        nc.sync.dma_start(rs0[:], rcp[idx : idx + 1, :])
                        rb = prb.tile([DH, QG], F32, tag="rb", bufs=8)
                        nc.gpsimd.partition_broadcast(rb[:], rs0[:])
                        rbs.append(rb)
                    for hp in range(NPAIR):
                        for j in range(2):
                            idx = 2 * hp + j
                            if j == 0:
                                nc.vector.tensor_tensor(
                                    mrg[0:DH, hp, :], av_all[0:DH, idx, :],
                                    rbs[idx][:], MULT,
                                )
                            else:
                                odt = prb.tile([DH, QG], BF16, tag="odt")
                                nc.vector.tensor_tensor(
                                    odt[:], av_all[0:DH, idx, :], rbs[idx][:],
                                    MULT,
                                )
                                nc.sync.dma_start(mrg[DH:128, hp, :], odt[:])
                return mrg, odts

            # Software pipeline: attention(t) drains a fill queue holding
            # proj(t+1)/proj(t+2) pieces and outproj(t-1) pieces, so the
            # PE's spare per-kt cycles absorb them and the Act engine never
            # waits on a dedicated projection phase.
            qst = pq.tile([128, NPAIR, QG], BF16, tag="qst", name="qst")
            p0 = proj_pieces(0, xt0, qst)
            for it in p0[:32]:  # q + k chains (need only wq/wk/xt0)
                it()
            load_late_consts()
            for it in p0[32:]:  # v chains
                it()
            xt_next = load_xt(1)
            qst_next = pq.tile([128, NPAIR, QG], BF16, tag="qst", name="qst")
            fill.extend((1, it) for it in proj_pieces(1, xt_next, qst_next))
            for tcn in range(NQG):
                drain_fill(tcn)
                mrg, odts = attention(tcn, qst)
                qst = qst_next
                if tcn + 2 < NQG:
                    xt_next = load_xt(tcn + 2)
                    qst_next = pq.tile(
                        [128, NPAIR, QG], BF16, tag="qst", name="qst"
                    )
                    fill.extend(
                        (tcn + 2, it)
                        for it in proj_pieces(tcn + 2, xt_next, qst_next)
                    )
                fill.extend(
                    (tcn + 2, it) for it in outproj_pieces(tcn, mrg, odts)
                )
            while fill:
                fill.pop(0)[1]()
    nc.compile()
    return nc


_NC_CACHE = None


def _get_nc():
    global _NC_CACHE
    if _NC_CACHE is None:
        _NC_CACHE = build_nc()
    return _NC_CACHE


def make_in_maps(x, Wq, Wk, Wv, Wo):
    x = np.asarray(x, dtype=np.float32)
    Wq = np.asarray(Wq, dtype=np.float32)
    Wk = np.asarray(Wk, dtype=np.float32)
    Wv = np.asarray(Wv, dtype=np.float32)
    Wo = np.asarray(Wo, dtype=np.float32)
    tri = np.triu(np.ones((128, 128), dtype=np.float32)).astype(BF)
    in_maps = []
    for c in range(8):
        b, g = divmod(c, 2)
        rows = slice(HG * g, HG * (g + 1))
        xT = x[b].T.reshape(NDC, 128, NQG, QG).transpose(2, 1, 0, 3)
        wq = Wq[rows].T.reshape(NDC, 128, HG).transpose(1, 0, 2)
        wk = Wk[rows].T.reshape(NDC, 128, HG).transpose(1, 0, 2)
        wv = Wv[rows].T.reshape(NDC, 128, HG).transpose(1, 0, 2)
        wo = Wo[:, rows].T.reshape(NPAIR, 128, D).transpose(1, 0, 2)
        in_maps.append(
            {
                "xT": np.ascontiguousarray(xT).astype(BF),
                "wq": np.ascontiguousarray(wq).astype(BF),
                "wk": np.ascontiguousarray(wk).astype(BF),
                "wv": np.ascontiguousarray(wv).astype(BF),
                "wo": np.ascontiguousarray(wo).astype(BF),
                "tri": tri,
            }
        )
    return in_maps


def run(x, Wq, Wk, Wv, Wo, trace=False, **spmd_kwargs):
    nc = _get_nc()
    in_maps = make_in_maps(x, Wq, Wk, Wv, Wo)
    res = run_bass_kernel_spmd(
        nc, in_maps, core_ids=list(range(8)), trace=trace, **spmd_kwargs
    )
    parts = [np.asarray(r["y"]).astype(np.float32) for r in res.results]
    y = np.stack([parts[2 * b] + parts[2 * b + 1] for b in range(B)])
    return y, res


def kernel(x, Wq, Wk, Wv, Wo):
    y, _ = run(x, Wq, Wk, Wv, Wo, trace=False)
    return y



# revision 43
# speedup vs baseline: 1.0012x; 1.0012x over previous
"""Trainium2 Bass kernel for nn_ChunkedAttention (B=4, T=4096, D=1024, H=16, dh=64).

Sharding: 8 cores = 4 batches x 2 head-groups (8 heads each). Host sums the
two partial out-projections per batch in fp32.

All inputs bf16, pre-shuffled host-side to [partition, chunk, free] so every
DMA is 128 contiguous 8KB descriptors. The attention inner loop runs in
2-kt blocks: both k-tiles' row-tiled (banded K=64) score pairs issue
back-to-back — banded pairs overlap on the PE and the ~100ns tile-mode
entry cost is paid once per block — then the previous block's four
full-array AV matmuls (exit cost paid once). Projection/out-projection work
is queued as 2-matmul fill pieces with deadlines (proj(t) before
attention(t), outproj(q) before attention(q+2)) and drained into the PE's
spare cycles between a block's scores and AVs, so the softmax (Act) engine
is never starved by a bulk projection phase and leftovers roll into later
(larger) groups' slack. The softmax denominator rides the AV matmuls as a
65th ones-column; reciprocals use the fast approximate DVE op; the last
group normalizes per head pair immediately so the final out-projection
isn't serialized behind the whole group.
"""

import os
import sys

import numpy as np

for _p in ("/opt/trn_rl_repo",):
    if _p not in sys.path and os.path.isdir(_p):
        sys.path.insert(0, _p)

import ml_dtypes

import concourse.bass as bass
import concourse.mybir as mybir
import concourse.tile as tile
from concourse.bacc import Bacc
from concourse.bass_utils import run_bass_kernel_spmd

F32 = mybir.dt.float32
BF16 = mybir.dt.bfloat16
EXP = mybir.ActivationFunctionType.Exp
MULT = mybir.AluOpType.mult

B, T, D = 4, 4096, 1024
HG = 512          # head-group width per core (8 heads x 64)
NH, DH = 8, 64    # heads per core, head dim
NPAIR = 4         # head pairs per core
QG = 512          # query-group width
NQG = T // QG     # 8
NKT = T // 128    # 32 k-tiles
NDC = D // 128    # 8 d_model chunks
SCALE = 1.0 / np.sqrt(DH)  # 0.125

BF = ml_dtypes.bfloat16


def build_nc():
    nc = Bacc()
    # x^T pre-shuffled host-side to [tcn, p, dc, u] so each per-chunk DMA is
    # 128 descriptors of 8KB (contiguous per partition) instead of 1024x1KB
    xT_d = nc.dram_tensor("xT", [NQG, 128, NDC, QG], BF16, kind="ExternalInput")
    # weights pre-shuffled host-side to [p, dc, h] so each DMA is 128
    # descriptors of 8KB (contiguous per partition) instead of 1024x1KB
    wq_d = nc.dram_tensor("wq", [128, NDC, HG], BF16, kind="ExternalInput")
    wk_d = nc.dram_tensor("wk", [128, NDC, HG], BF16, kind="ExternalInput")
    wv_d = nc.dram_tensor("wv", [128, NDC, HG], BF16, kind="ExternalInput")
    wo_d = nc.dram_tensor("wo", [128, NPAIR, D], BF16, kind="ExternalInput")
    tri_d = nc.dram_tensor("tri", [128, 128], BF16, kind="ExternalInput")
    y_d = nc.dram_tensor("y", [T, D], BF16, kind="ExternalOutput")

    with tile.TileContext(nc) as tc_:
        with (
            tc_.tile_pool(name="const", bufs=1) as pconst,
            tc_.tile_pool(name="pxt", bufs=2) as pxt,
            tc_.tile_pool(name="pq", bufs=3) as pq,
            tc_.tile_pool(name="pet", bufs=6) as pet,
            tc_.tile_pool(name="pnrm", bufs=2) as pnrm,
            tc_.tile_pool(name="prb", bufs=3) as prb,
            tc_.tile_pool(name="psS", bufs=2, space="PSUM") as psS,
            tc_.tile_pool(name="psAV", bufs=2, space="PSUM") as psAV,
            tc_.tile_pool(name="psM", bufs=2, space="PSUM") as psM,
        ):
            kt_sb = pconst.tile([128, NPAIR, T], BF16, tag="kt")
            v_sb = pconst.tile([128, NKT, NH, DH + 1], BF16, tag="v")
            tri_sb = pconst.tile([128, 128], BF16, tag="tri")
            wq_sb = pconst.tile([128, NDC, HG], BF16, tag="wq")
            wk_sb = pconst.tile([128, NDC, HG], BF16, tag="wk")
            wv_sb = pconst.tile([128, NDC, HG], BF16, tag="wv")
            wo_sb = pconst.tile([128, NPAIR, D], BF16, tag="wo")

            def load_xt_first(tcn):
                # split into 2-dc parts so the first proj pieces (which read
                # 2 dc each) start after ~0.25MB lands, not the full 1MB
                xt = pxt.tile([128, NDC, QG], BF16, tag="xt", name="xt")
                for d0 in range(0, NDC, 2):
                    nc.sync.dma_start(
                        xt[:, d0 : d0 + 2, :], xT_d[tcn, :, d0 : d0 + 2, :]
                    )
                return xt

            # startup DMA priority: the 5MB of inputs are HBM-bandwidth
            # bound (~12us), so emit only what the first q/k chains need
            # first; wv/tri/wo descriptors queue behind them
            xt0 = load_xt_first(0)
            for d0 in range(0, NDC, 2):
                nc.sync.dma_start(
                    wq_sb[:, d0 : d0 + 2, :], wq_d[:, d0 : d0 + 2, :]
                )
            nc.sync.dma_start(wk_sb[:], wk_d[:])
            nc.gpsimd.memset(v_sb[:, :, :, DH : DH + 1], 1.0)

            def load_late_consts():
                nc.sync.dma_start(wv_sb[:], wv_d[:])
                nc.sync.dma_start(tri_sb[:], tri_d[:])
                nc.sync.dma_start(wo_sb[:], wo_d[:])

            def load_xt(tcn):
                xt = pxt.tile([128, NDC, QG], BF16, tag="xt", name="xt")
                nc.sync.dma_start(xt[:], xT_d[tcn])
                return xt

            # chains are emitted as 2-matmul pieces (~430ns of PE each) so
            # the fill queue never blocks the PE long enough to starve the
            # Act engine between score matmuls
            def q_chain_pieces(tcn, xt, qst, hp):
                st = {}

                def piece(d0):
                    if d0 == 0:
                        st["ps"] = psM.tile([128, QG], F32, tag="mm", name="pq_ps")
                    for dc in range(d0, d0 + 2):
                        nc.tensor.matmul(
                            st["ps"][:],
                            wq_sb[:, dc, hp * 128 : (hp + 1) * 128],
                            xt[:, dc, :],
                            start=(dc == 0),
                            stop=(dc == NDC - 1),
                        )
                    if d0 == NDC - 2:
                        nc.vector.tensor_copy(qst[:, hp, :], st["ps"][:])

                return [lambda d0=d0: piece(d0) for d0 in range(0, NDC, 2)]

            def k_chain_pieces(tcn, xt, hp):
                st = {}

                def piece(d0):
                    if d0 == 0:
                        st["ps"] = psM.tile([128, QG], F32, tag="mm", name="pk_ps")
                    for dc in range(d0, d0 + 2):
                        nc.tensor.matmul(
                            st["ps"][:],
                            wk_sb[:, dc, hp * 128 : (hp + 1) * 128],
                            xt[:, dc, :],
                            start=(dc == 0),
                            stop=(dc == NDC - 1),
                        )
                    if d0 == NDC - 2:
                        nc.vector.tensor_copy(
                            kt_sb[:, hp, tcn * QG : (tcn + 1) * QG], st["ps"][:]
                        )

                return [lambda d0=d0: piece(d0) for d0 in range(0, NDC, 2)]

            def v_chain_pieces(tcn, xt, ts):
                st = {}

                def piece(d0):
                    if d0 == 0:
                        st["ps"] = psM.tile([128, QG], F32, tag="mm", name="pv_ps")
                    for dc in range(d0, d0 + 2):
                        nc.tensor.matmul(
                            st["ps"][:],
                            xt[:, dc, ts * 128 : (ts + 1) * 128],
                            wv_sb[:, dc, :],
                            start=(dc == 0),
                            stop=(dc == NDC - 1),
                        )
                    if d0 == NDC - 2:
                        ktg = tcn * 4 + ts
                        nc.vector.tensor_copy(
                            v_sb[:, ktg, :, 0:DH],
                            st["ps"].rearrange("p (h d) -> p h d", h=NH),
                        )

                return [lambda d0=d0: piece(d0) for d0 in range(0, NDC, 2)]

            def proj_pieces(tcn, xt, qst):
                items = []
                for hp in range(NPAIR):
                    items.extend(q_chain_pieces(tcn, xt, qst, hp))
                for hp in range(NPAIR):
                    items.extend(k_chain_pieces(tcn, xt, hp))
                for ts in range(4):
                    items.extend(v_chain_pieces(tcn, xt, ts))
                return items

            def op_pieces(qg, mrg, qc, half, odts=None):
                st = {}

                def finish(ps):
                    yt = prb.tile([128, 512], BF16, tag="yt")
                    nc.vector.tensor_copy(yt[:], ps[:])
                    nc.sync.dma_start(
                        y_d[
                            qg * QG + qc * 128 : qg * QG + (qc + 1) * 128,
                            half * 512 : (half + 1) * 512,
                        ],
                        yt[:],
                    )

                def piece(h0):
                    if h0 == 0:
                        st["ps"] = psM.tile([128, 512], F32, tag="mm", name="op")
                    for hp in range(h0, h0 + 2):
                        nc.tensor.matmul(
                            st["ps"][:],
                            mrg[:, hp, qc * 128 : (qc + 1) * 128],
                            wo_sb[:, hp, half * 512 : (half + 1) * 512],
                            start=(hp == 0),
                            stop=(hp == NPAIR - 1),
                        )
                    if h0 == NPAIR - 2:
                        finish(st["ps"])

                def piece_banded(h0):
                    # last group: the odd head's normalized output never got
                    # the partition-shift DMA into mrg rows 64-127; contract
                    # it from partitions 0-63 as a K=64 row tile at array
                    # rows 64-127 instead (keeps the final out-projection
                    # off the DMA latency path)
                    if h0 == 0:
                        st["ps"] = psM.tile([128, 512], F32, tag="mm", name="op")
                    for hp in range(h0, h0 + 2):
                        nc.tensor.matmul(
                            st["ps"][:],
                            mrg[0:64, hp, qc * 128 : (qc + 1) * 128],
                            wo_sb[0:64, hp, half * 512 : (half + 1) * 512],
                            start=(hp == 0),
                            stop=False,
                            tile_position=(0, 0),
                        )
                        nc.tensor.matmul(
                            st["ps"][:],
                            odts[hp][:, qc * 128 : (qc + 1) * 128],
                            wo_sb[64:128, hp, half * 512 : (half + 1) * 512],
                            start=False,
                            stop=(hp == NPAIR - 1),
                            tile_position=(64, 0),
                        )
                    if h0 == NPAIR - 2:
                        finish(st["ps"])

                pc = piece_banded if odts is not None else piece
                return [lambda h0=h0: pc(h0) for h0 in range(0, NPAIR, 2)]

            def outproj_pieces(qg, mrg, odts=None):
                odts = odts or None  # only the last group populates it
                items = []
                for qc in range(4):
                    for half in range(2):
                        items.extend(op_pieces(qg, mrg, qc, half, odts))
                return items

            # deferred PE work as (deadline, fn): fn must run before
            # attention(deadline) starts (proj(t) feeds attention(t);
            # outproj(q) must finish before mrg(q)'s pool slot recycles at
            # attention(q+2)). Leftovers roll forward into later groups'
            # PE slack instead of stalling the Act engine in a bulk drain.
            fill = []

            def normalize_hp(qg, hp, av, av_all, mrg, odts):
                """Normalize one head pair with a fast approximate
                reciprocal (saves ~3.4us of iterative DVE divide per head
                pair on the final group's latency-critical tail)."""
                for j in (1, 0):  # j=1 first: its result rides a DMA, so
                    # start that leg of the chain as early as possible
                    idx = 2 * hp + j
                    rs0b = prb.tile([1, QG], BF16, tag="rs0b", bufs=2)
                    nc.sync.dma_start(rs0b[:], av_all[DH : DH + 1, idx, :])
                    rs0 = prb.tile([1, QG], F32, tag="rs0")
                    nc.vector.tensor_copy(rs0[:], rs0b[:])
                    nc.vector.reciprocal_approx_fast(rs0[:], rs0[:])
                    rb = prb.tile([DH, QG], F32, tag="rb", bufs=8)
                    nc.gpsimd.partition_broadcast(rb[:], rs0[:])
                    if j == 0:
                        nc.vector.tensor_tensor(
                            mrg[0:DH, hp, :], av_all[0:DH, idx, :], rb[:], MULT
                        )
                    else:
                        # (a banded K=64 out-projection from partitions 0-63
                        # at tile row 64 would skip this DMA, but walrus
                        # codegen rejects lhsT/tile_position partition
                        # mismatch — the partition shift must ride a DMA)
                        odt = prb.tile([DH, QG], BF16, tag="odt7", bufs=4)
                        nc.vector.tensor_tensor(
                            odt[:], av_all[0:DH, idx, :], rb[:], MULT
                        )
                        nc.sync.dma_start(mrg[DH:128, hp, :], odt[:])

            def drain_fill(deadline):
                while fill and fill[0][0] <= deadline:
                    fill.pop(0)[1]()

            def attention(qg, qst):
                """Causal attention + softmax normalize for query group qg.

                Drains the fill queue (projection pieces for future
                T-chunks, previous group's out-projection pieces) into the
                PE's spare per-kt cycles.
                """
                ktmax = 4 * (qg + 1)
                n_kt = NPAIR * ktmax
                odts = {}
                av_all = pnrm.tile([DH + 1, 2 * NPAIR, QG], BF16, tag="avsb")
                sum_sb = pnrm.tile([2 * NPAIR, QG], BF16, tag="sums")
                mrg = pnrm.tile([128, NPAIR, QG], BF16, tag="mrg")
                # pace fill pops over the group's kt iterations, starting a
                # couple of kts in (lets input DMAs land first); leftovers
                # roll to later (larger) groups
                kt_count = 0

                for hp in range(NPAIR):
                    av = [
                        psAV.tile([DH + 1, QG], F32, tag="av", name=f"av{j}")
                        for j in range(2)
                    ]

                    def emit_av(e_t, kt):
                        diag = kt - 4 * qg
                        dlt = 128 * diag if diag >= 0 else 0
                        for j in range(2):
                            nc.tensor.matmul(
                                av[j][:, dlt:],
                                v_sb[:, kt, 2 * hp + j, :],
                                e_t[:, j, dlt:],
                                start=(kt == 0),
                                stop=(kt == ktmax - 1),
                            )

                    # 2-kt blocks: both kts' banded score pairs issue
                    # back-to-back (bands overlap, and the ~100ns tile-mode
                    # entry cost is paid once per block, not per kt), then
                    # the previous block's 4 full-array AV matmuls (paying
                    # the banded->full exit cost once). exps of the current
                    # block run on Act while the previous block's AVs run
                    # on the PE.
                    pend = []
                    for ktb in range(0, ktmax, 2):
                        kts = (ktb, ktb + 1)
                        ets = []
                        for kt in kts:
                            diag = kt - 4 * qg
                            dlt = 128 * diag if diag >= 0 else 0
                            s_t = psS.tile([128, 2, QG], F32, tag="st")
                            for j in range(2):
                                nc.tensor.matmul(
                                    s_t[:, j, dlt:],
                                    kt_sb[
                                        64 * j : 64 * (j + 1),
                                        hp,
                                        kt * 128 : (kt + 1) * 128,
                                    ],
                                    qst[64 * j : 64 * (j + 1), hp, dlt:],
                                    start=True,
                                    stop=True,
                                    tile_position=(64 * j, 0),
                                )
                            ets.append((s_t, dlt, diag))
                        new_pend = []
                        for kt, (s_t, dlt, diag) in zip(kts, ets):
                            e_t = pet.tile([128, 2, QG], BF16, tag="exps")
                            nc.scalar.activation(
                                e_t[:, :, dlt:], s_t[:, :, dlt:], EXP,
                                scale=SCALE,
                            )
                            if diag >= 0:
                                for j in range(2):
                                    nc.vector.tensor_tensor(
                                        e_t[:, j, dlt : dlt + 128],
                                        e_t[:, j, dlt : dlt + 128],
                                        tri_sb[:],
                                        MULT,
                                    )
                            new_pend.append((e_t, kt))
                        kt_count += 2
                        if fill and kt_count >= (3 if qg >= 2 else 1):
                            # pop fill between the scores and the AVs: the
                            # banded->full transition cost is lower into a
                            # proj piece than into an AV, and the pend AVs
                            # have no consumer waiting on them
                            slots = n_kt - kt_count
                            npop = max(1, -(-(2 * len(fill)) // max(1, slots)))
                            for _ in range(min(npop, 4, len(fill))):
                                fill.pop(0)[1]()
                        for p in pend:
                            emit_av(*p)
                        pend = new_pend
                    # row end: the last exps have no following score work, so
                    # feed the PE fill pieces while the Act engine drains them
                    for _ in range(min(3, len(fill))):
                        fill.pop(0)[1]()
                    for p in pend:
                        emit_av(*p)
                    # drain PSUM to SBUF so the next head pair can accumulate
                    last = qg == NQG - 1
                    for j in range(2):
                        idx = 2 * hp + j
                        nc.vector.tensor_copy(av_all[:, idx, :], av[j][:])
                        if not last:
                            nc.sync.dma_start(
                                sum_sb[idx : idx + 1, :],
                                av_all[DH : DH + 1, idx, :],
                            )
                    if last:
                        # last group: normalize per head pair immediately so
                        # the final out-projection isn't serialized at the end
                        normalize_hp(qg, hp, av, av_all, mrg, odts)

                if qg < NQG - 1:
                    # batched reciprocal of all 8 softmax-sum rows, then
                    # broadcasts (GpSimd) overlapping multiplies (DVE)
                    rcp = pnrm.tile([2 * NPAIR, QG], F32, tag="rcp")
                    nc.vector.tensor_copy(rcp[:], sum_sb[:])
                    nc.vector.reciprocal_approx_fast(rcp[:], rcp[:])
                    rbs = []
                    for idx in range(2 * NPAIR):
                        rs0 = prb.tile([1, QG], F32, tag="rs0")
                        nc.sync.dma_start(rs0[:], rcp[idx : idx + 1, :])
                        rb = prb.tile([DH, QG], F32, tag="rb", bufs=8)
                        nc.gpsimd.partition_broadcast(rb[:], rs0[:])
                        rbs.append(rb)
                    for hp in range(NPAIR):
                        for j in range(2):
                            idx = 2 * hp + j
                            if j == 0:
                                nc.vector.tensor_tensor(
                                    mrg[0:DH, hp, :], av_all[0:DH, idx, :],
                                    rbs[idx][:], MULT,
                                )
                            else:
                                odt = prb.tile([DH, QG], BF16, tag="odt")
                                nc.vector.tensor_tensor(
                                    odt[:], av_all[0:DH, idx, :], rbs[idx][:],
                                    MULT,
                                )
                                nc.sync.dma_start(mrg[DH:128, hp, :], odt[:])
                return mrg, odts

            # Software pipeline: attention(t) drains a fill queue holding
            # proj(t+1)/proj(t+2) pieces and outproj(t-1) pieces, so the
            # PE's spare per-kt cycles absorb them and the Act engine never
            # waits on a dedicated projection phase.
            qst = pq.tile([128, NPAIR, QG], BF16, tag="qst", name="qst")
            p0 = proj_pieces(0, xt0, qst)
            for it in p0[:32]:  # q + k chains (need only wq/wk/xt0)
                it()
            load_late_consts()
            for it in p0[32:]:  # v chains
                it()
            xt_next = load_xt(1)
            qst_next = pq.tile([128, NPAIR, QG], BF16, tag="qst", name="qst")
            fill.extend((1, it) for it in proj_pieces(1, xt_next, qst_next))
            for tcn in range(NQG):
                drain_fill(tcn)
                mrg, odts = attention(tcn, qst)
                qst = qst_next
                if tcn + 2 < NQG:
                    xt_next = load_xt(tcn + 2)
                    qst_next = pq.tile(
                        [128, NPAIR, QG], BF16, tag="qst", name="qst"
                    )
                    fill.extend(
                        (tcn + 2, it)
                        for it in proj_pieces(tcn + 2, xt_next, qst_next)
                    )
                fill.extend(
                    (tcn + 2, it) for it in outproj_pieces(tcn, mrg, odts)
                )
            while fill:
                fill.pop(0)[1]()
    nc.compile()
    return nc


_NC_CACHE = None


def _get_nc():
    global _NC_CACHE
    if _NC_CACHE is None:
        _NC_CACHE = build_nc()
    return _NC_CACHE


def make_in_maps(x, Wq, Wk, Wv, Wo):
    x = np.asarray(x, dtype=np.float32)
    Wq = np.asarray(Wq, dtype=np.float32)
    Wk = np.asarray(Wk, dtype=np.float32)
    Wv = np.asarray(Wv, dtype=np.float32)
    Wo = np.asarray(Wo, dtype=np.float32)
    tri = np.triu(np.ones((128, 128), dtype=np.float32)).astype(BF)
    in_maps = []
    for c in range(8):
        b, g = divmod(c, 2)
        rows = slice(HG * g, HG * (g + 1))
        xT = x[b].T.reshape(NDC, 128, NQG, QG).transpose(2, 1, 0, 3)
        wq = Wq[rows].T.reshape(NDC, 128, HG).transpose(1, 0, 2)
        wk = Wk[rows].T.reshape(NDC, 128, HG).transpose(1, 0, 2)
        wv = Wv[rows].T.reshape(NDC, 128, HG).transpose(1, 0, 2)
        wo = Wo[:, rows].T.reshape(NPAIR, 128, D).transpose(1, 0, 2)
        in_maps.append(
            {
                "xT": np.ascontiguousarray(xT).astype(BF),
                "wq": np.ascontiguousarray(wq).astype(BF),
                "wk": np.ascontiguousarray(wk).astype(BF),
                "wv": np.ascontiguousarray(wv).astype(BF),
                "wo": np.ascontiguousarray(wo).astype(BF),
                "tri": tri,
            }
        )
    return in_maps


def run(x, Wq, Wk, Wv, Wo, trace=False, **spmd_kwargs):
    nc = _get_nc()
    in_maps = make_in_maps(x, Wq, Wk, Wv, Wo)
    res = run_bass_kernel_spmd(
        nc, in_maps, core_ids=list(range(8)), trace=trace, **spmd_kwargs
    )
    parts = [np.asarray(r["y"]).astype(np.float32) for r in res.results]
    y = np.stack([parts[2 * b] + parts[2 * b + 1] for b in range(B)])
    return y, res


def kernel(x, Wq, Wk, Wv, Wo):
    y, _ = run(x, Wq, Wk, Wv, Wo, trace=False)
    return y

